# revision 42
# baseline (speedup 1.0000x reference)
"""Trainium2 Bass kernel for nn_Att6 (attention-pooling block).

Computes, for each batch b:
    ht  = tanh(t[b] @ wt)                     (T, H)
    c   = tanh(a[b] @ wa) * tanh(b[b] @ wb) * wh[:, 0]        (H,)
    s   = ht @ c                              (T,)   scores
    att = softmax(s) * mask; att /= sum(att)  (T,)
    out = att @ t[b]                          (D,)

Sharding: data-parallel over batch B=32 across 8 NeuronCores (4 batches
per core), weights replicated.

Precision/layout strategy:
  - t is shipped twice from the host: native bf16 (tau on partitions,
    for pooling) and pre-transposed fp8e4m3 (d on partitions, for the
    big matmul).  No transposes on device at all.
  - mm1 (t @ wt) runs in fp8 DoubleRow perf mode (2 k-tiles per
    instruction, ~1.55x the bf16 rate incl. LDWEIGHTS overhead), with a
    1024-wide moving operand (2-bank PSUM tile) to halve the
    per-instruction LDWEIGHTS/decode overhead.  wt is pre-scaled by
    WT_SCALE on the host so its values sit in fp8's normal range; the
    tanh activation divides the scale back out.
  - mm2 (scores) also runs fp8 DoubleRow: tanh emits fp8 hT pairs, and
    the c vector is pre-scaled by CSCALE (folded into wh on the host)
    so fp8 quantization of c is benign; the exp activation divides
    CSCALE back out.  Host-emulated end-to-end rel(max) = 1.53e-2.
  - pooling stays bf16 (fp8 pooling would be 2.8e-2 > the 2e-2 gate).
"""

import sys

sys.path.insert(0, "/opt/trn_rl_repo")

import numpy as np

import bass_rust
import concourse.bass as bass
import concourse.tile as tile
from concourse import mybir
from concourse.masks import make_identity

F32 = mybir.dt.float32
F32R = mybir.dt.float32r
BF16 = mybir.dt.bfloat16
F8 = mybir.dt.float8e4
AF = mybir.ActivationFunctionType
AX = mybir.AxisListType
DR = mybir.MatmulPerfMode.DoubleRow

WT_SCALE = 32.0
CSCALE = 64.0

N_CORES = 8
B, T, D, H = 32, 2048, 1024, 1024
BL = B // N_CORES            # batches per core
TCH = 512                    # tau-chunk (pool/score granularity)
NTCH = T // TCH              # 4 chunks per batch
NPAIR = NTCH // 2            # mm1 runs on chunk pairs (1024-wide moving)
NTT = TCH // 128             # 4 tau-tiles per chunk
KD = D // 128                # 8 contraction chunks over D
KH = H // 128                # 8 chunks over H

# Ablation knobs (timing experiments only — wrong results when not default):
# pool: emit pooling matmuls; mm2: # of hh-pair score matmuls (4=all);
# mm1_k: # of DR k2 matmuls per hh (4=all)
ABLATE_DEFAULT = {"pool": True, "mm2": 4, "mm1_k": 4}
ABLATE = dict(ABLATE_DEFAULT)


def split_sync_waits(nc, max_waits=1):
    """This container's walrus accepts only one sem-wait per instruction.
    Move extra waits onto same-engine NOPs inserted immediately before."""
    n_new = 0
    for f in nc.m.functions:
        for bb in f.blocks:
            new = []
            for inst in bb.instructions:
                si = inst.sync_info
                waits = list(si.on_wait) if (si and si.on_wait) else []
                if len(waits) > max_waits:
                    extra, keep = waits[:-max_waits], waits[-max_waits:]
                    for w in extra:
                        nop = bass_rust.InstNoOp(
                            name=f"{inst.name}-sw{n_new}", ins=[], outs=[])
                        nop.engine = inst.engine
                        nop.sync_info = mybir.SyncInfo(on_wait=[w], on_update=[])
                        new.append(nop)
                        n_new += 1
                    si.on_wait = keep
                new.append(inst)
            bb.instructions[:] = new
    return n_new


def build_nc(split_waits=True, reps=1):
    nc = bass.Bass()
    t_in = nc.declare_dram_parameter("t", [BL, T, D], BF16, isOutput=False)
    # host-transposed fp8 copy: t8[b, p, k, tau] = fp8(t[b, tau, k*128+p])
    t8_in = nc.declare_dram_parameter("t8", [BL, 128, KD, T], F8,
                                      isOutput=False)
    a_in = nc.declare_dram_parameter("a", [BL, D], F32, isOutput=False)
    b_in = nc.declare_dram_parameter("b", [BL, D], F32, isOutput=False)
    m_in = nc.declare_dram_parameter("mask", [BL, T], F32, isOutput=False)
    wt_in = nc.declare_dram_parameter("wt", [D, H], F32, isOutput=False)
    wa_in = nc.declare_dram_parameter("wa", [D, H], F32, isOutput=False)
    wb_in = nc.declare_dram_parameter("wb", [D, H], F32, isOutput=False)
    wh_in = nc.declare_dram_parameter("wh", [H], F32, isOutput=False)
    out_d = nc.declare_dram_parameter("out", [BL, D], F32, isOutput=True)

    with tile.TileContext(nc) as tc:
        _body(nc, tc, t_in, t8_in, a_in, b_in, m_in, wt_in, wa_in, wb_in,
              wh_in, out_d, reps)
    if split_waits:
        split_sync_waits(nc)
    return nc


def _body(nc, tc, t_in, t8_in, a_in, b_in, m_in, wt_in, wa_in, wb_in, wh_in,
          out_d, reps):
    with (
        tc.tile_pool(name="const", bufs=1) as const,
        tc.tile_pool(name="wts", bufs=1) as wts,
        tc.tile_pool(name="wtstage", bufs=2) as wtstage,
        tc.tile_pool(name="wab", bufs=4) as wab,
        tc.tile_pool(name="small", bufs=1) as small,
        tc.tile_pool(name="tch", bufs=6) as tch,
        tc.tile_pool(name="t8b", bufs=2) as t8bp,
        tc.tile_pool(name="hT", bufs=3) as hTp,
        tc.tile_pool(name="rows", bufs=2) as rows,
        tc.tile_pool(name="rowsm", bufs=2) as rowsm,
        # PSUM budget (8 banks): mm1 3x1 + tr 1 + s 2x1 + o 2x1 = 8
        tc.tile_pool(name="ps_mm", bufs=3, space="PSUM") as ps_mm,
        tc.tile_pool(name="ps_tr", bufs=1, space="PSUM") as ps_tr,
        tc.tile_pool(name="ps_s", bufs=1, space="PSUM") as ps_sp,
        tc.tile_pool(name="ps_o", bufs=2, space="PSUM") as ps_op,
    ):
        ident = const.tile([128, 128], F32)
        make_identity(nc, ident)

        # tiny vector loads + transposes first so PE work exists early
        vT = {}
        for name, vec_in in (("a", a_in), ("b", b_in)):
            v_nat = small.tile([BL, D], F32, tag=f"v{name}")
            nc.sync.dma_start(out=v_nat, in_=vec_in[:, :])
            vT_sb = small.tile([128, KD, BL], F32R, tag=f"vT{name}")
            vT[name] = vT_sb
            for k in range(KD):
                ps = ps_tr.tile([128, BL], F32, tag="tr")
                nc.tensor.transpose(
                    ps, v_nat[:, k * 128:(k + 1) * 128], ident[:BL, :BL])
                nc.vector.tensor_copy(vT_sb[:, k, :], ps)

        def emit_chunk_dma(b, j, halves=1):
            t_nat = tch.tile([128, NTT, D], BF16, tag="tch", name="t_nat")
            hs = NTT // halves
            for h0 in range(0, NTT, hs):
                nc.sync.dma_start(
                    out=t_nat[:, h0:h0 + hs, :],
                    in_=t_in[b, (j * NTT + h0) * 128:(j * NTT + h0 + hs) * 128, :]
                    .rearrange("(tt p) d -> p tt d", p=128))
            return t_nat

        def emit_batch_t8(b):
            # host-pretransposed fp8 t: one big contiguous DMA per batch,
            # on the Activation HWDGE queue so it never queues behind the
            # native chunk stream
            t8b = t8bp.tile([128, KD, T], F8, tag="t8b", name="t8b")
            nc.scalar.dma_start(out=t8b, in_=t8_in[b])
            return t8b

        t8_batch = {0: emit_batch_t8(0)}
        nat_pre = {(0, 0): emit_chunk_dma(0, 0)}

        # weight DMA order: per h-chunk, wa/wb pair (phase 0 consumes them
        # early) interleaved with the matching wt h-slice
        w_tiles = {}
        wt_sb = wts.tile([128, KD, H], F8)
        for hh in range(KH):
            # weight DMAs ride the Activation HWDGE queue so the startup
            # weight flood never shares a queue with the chunk stream
            for name, w_in in (("a", wa_in), ("b", wb_in)):
                w_sb = wab.tile(
                    [128, KD, 128], F32R, tag="wsb", name=f"w{name}{hh}")
                nc.scalar.dma_start(
                    out=w_sb,
                    in_=w_in[:, hh * 128:(hh + 1) * 128]
                    .bitcast(F32R).rearrange("(k p) h -> p k h", p=128))
                w_tiles[(name, hh)] = w_sb
            # one-time fp8 weight quantization via a small rotating f32
            # stage (amortized across reps)
            wstage = wtstage.tile([128, KD, 128], F32R, tag="ws",
                                  name=f"ws{hh}")
            nc.scalar.dma_start(
                out=wstage,
                in_=wt_in[:, hh * 128:(hh + 1) * 128]
                .bitcast(F32R).rearrange("(k p) h -> p k h", p=128))
            nc.vector.tensor_copy(
                wt_sb[:, :, hh * 128:(hh + 1) * 128], wstage)
            if hh == 2:
                # slip batch0-chunk1's t DMA into the weight stream so its
                # data is resident when chunk0's compute finishes
                nat_pre[(0, 1)] = emit_chunk_dma(0, 1)

        whT_sb = const.tile([128, KH], F32)
        nc.scalar.dma_start(out=whT_sb,
                            in_=wh_in.rearrange("(k p) -> p k", p=128))

        # ---- phase 0 (h-chunked, interleaved into the first pair's hh
        # loop): c = tanh(a@wa) * tanh(b@wb) * wh  (wh pre-scaled by
        # CSCALE on the host); cT8 is the fp8 copy mm2 consumes ----
        cT_sb = small.tile([128, KH, BL], F32)
        # exact-1.0 fp8 stationary for the mm2 partition-sum (padded to
        # 16 cols so the DR stationary's k-tile step is 16B)
        ones8 = const.tile([128, 2, 16], F8, tag="ones8")
        nc.vector.memset(ones8, 1.0)

        def emit_phase0_hh(hh):
            hv = {}
            for name in ("a", "b"):
                w_sb = w_tiles.pop((name, hh))
                ps = ps_tr.tile([128, BL], F32, tag="tr", name="p0")
                for k in range(KD):
                    nc.tensor.matmul(
                        ps, w_sb[:, k, :], vT[name][:, k, :],
                        start=(k == 0), stop=(k == KD - 1))
                hv[name] = wab.tile(
                    [128, BL], F32, tag=f"h{name}", name=f"h{name}")
                nc.scalar.activation(hv[name], ps, AF.Tanh)
            prod = wab.tile([128, BL], F32, tag="prod")
            nc.vector.tensor_mul(prod, hv["a"], hv["b"])
            nc.vector.tensor_mul(
                cT_sb[:, hh, :], prod,
                whT_sb[:, hh:hh + 1].to_broadcast([128, BL]))

        # ---- main loop over chunk PAIRS: mm1 (1024-wide fp8 DR) ->
        # tanh (fp8 out) -> mm2 (fp8 DR over hh pairs) -> exp -> partial
        # pooling accumulate.  No score-max subtraction: |s| <= ||wh||_1
        # ~ 36 << 88, so exp cannot overflow.
        seq = [(rep, b) for rep in range(reps) for b in range(BL)]
        deferred = []

        def flush_deferred():
            while deferred:
                deferred.pop(0)()

        def make_pool_partial(b, j, t_nat, att_b, ps_out, den_parts,
                              finalize):
            def fn():
                if not ABLATE["pool"]:
                    if finalize:
                        out_b = rows.tile([1, D], F32, tag="orow",
                                          name="out_b")
                        nc.vector.tensor_copy(out_b, att_b[:, :D])
                        nc.sync.dma_start(out=out_d[b:b + 1, :], in_=out_b)
                    return
                # transpose the 4 e-columns, accumulate the pooling matmul
                attT = rowsm.tile([128, NTT], BF16, tag="attT", name="attT")
                ps_a = ps_tr.tile([128, NTT], F32, tag="tr", name="ps_a")
                for tt in range(NTT):
                    i = j * NTT + tt
                    nc.tensor.transpose(
                        ps_a[:, tt:tt + 1],
                        att_b[:, i * 128:(i + 1) * 128], ident[:1, :1])
                nc.vector.tensor_copy(attT, ps_a)
                for dh in range(2):
                    for tt in range(NTT):
                        nc.tensor.matmul(
                            ps_out[dh], attT[:, tt:tt + 1],
                            t_nat[:, tt, dh * TCH:(dh + 1) * TCH],
                            start=(j == 0 and tt == 0),
                            stop=(j == NTCH - 1 and tt == NTT - 1),
                            skip_group_check=True)
                if finalize:
                    den = rowsm.tile([1, 1], F32, tag="den", name="den")
                    nc.vector.reduce_sum(
                        out=den, in_=den_parts[:, :], axis=AX.X)
                    rden = rowsm.tile([1, 1], F32, tag="rden", name="rden")
                    nc.vector.reciprocal(rden, den)
                    out_b = rows.tile([1, D], F32, tag="orow", name="out_b")
                    for dh in range(2):
                        nc.vector.tensor_scalar_mul(
                            out_b[:, dh * TCH:(dh + 1) * TCH], ps_out[dh],
                            rden)
                    nc.sync.dma_start(out=out_d[b:b + 1, :], in_=out_b)
            return fn

        masks = {}
        for idx, (rep, b) in enumerate(seq):
            if idx in masks:
                mask_b = masks.pop(idx)
            else:
                mask_b = rows.tile([1, T], F32, tag="mrow")
                nc.sync.dma_start(out=mask_b, in_=m_in[b:b + 1, :])
            att_b = rows.tile([1, T], F32, tag="arow")
            den_parts = rowsm.tile([1, NTCH], F32, tag="denp")
            ps_out = [ps_op.tile([1, TCH], F32, tag="o", name=f"o{dh}")
                      for dh in range(2)]
            t8b = t8_batch[idx]
            for jp in range(NPAIR):
                j0, j1 = 2 * jp, 2 * jp + 1
                t_nat = [nat_pre.pop((idx, j0)), nat_pre.pop((idx, j1))]

                # --- prefetches for upcoming chunks, queued ahead of this
                # pair's PE work (Tile keeps per-engine emission order)
                for dj in range(2):
                    nxt_j = 2 * jp + 2 + dj
                    if nxt_j < NTCH:
                        nkey = (idx, nxt_j)
                    else:
                        nkey = (idx + 1, nxt_j - NTCH)
                    if nkey[0] < len(seq) and nkey not in nat_pre:
                        nat_pre[nkey] = emit_chunk_dma(seq[nkey[0]][1],
                                                       nkey[1])
                if jp == 0 and idx + 1 < len(seq) \
                        and idx + 1 not in t8_batch:
                    t8_batch[idx + 1] = emit_batch_t8(seq[idx + 1][1])
                if jp == 1 and idx + 1 < len(seq):
                    mrow = rows.tile([1, T], F32, tag="mrow")
                    nc.sync.dma_start(
                        out=mrow,
                        in_=m_in[seq[idx + 1][1]:seq[idx + 1][1] + 1, :])
                    masks[idx + 1] = mrow
                if jp == NPAIR - 1:
                    t8_batch.pop(idx, None)

                ps_s = [ps_sp.tile([1, TCH], F32, tag=f"s{q}",
                                   name=f"s{q}") for q in range(2)]
                # mm2 runs 1.5 hh-PAIRs behind mm1 so the tanh + c-multiply
                # chain (Act then ~60%-busy DVE) has ~3 mm1 groups of slack
                # before mm2 reads the fp8 product
                mm2_pend = []

                def emit_mm2(pend, stop_p):
                    p, tiles = pend
                    if p >= ABLATE["mm2"]:
                        return
                    for q in range(2):
                        nc.tensor.matmul(
                            ps_s[q], ones8[:, :, 0:1], tiles[q],
                            start=(p == 0), stop=(p == stop_p),
                            perf_mode=DR, skip_group_check=True)

                for hh in range(KH):
                    nk2 = ABLATE["mm1_k"]
                    ps_h = [ps_mm.tile([128, TCH], F32, tag="mm1",
                                       name=f"mm1{q}") for q in range(2)]
                    for q, j in ((0, j0), (1, j1)):
                        sl = slice(j * TCH, (j + 1) * TCH)
                        for k2 in range(nk2):
                            nc.tensor.matmul(
                                ps_h[q],
                                wt_sb[:, 2 * k2:2 * k2 + 2,
                                      hh * 128:(hh + 1) * 128],
                                t8b[:, 2 * k2:2 * k2 + 2, sl],
                                start=(k2 == 0), stop=(k2 == nk2 - 1),
                                perf_mode=DR)
                    p = hh // 2
                    if idx == 0 and jp == 0:
                        # phase0(hh) must precede the c-multiply below,
                        # which consumes cT_sb[:, hh] in the same iteration
                        emit_phase0_hh(hh)
                    if hh % 2 == 0:
                        hT8 = [hTp.tile([128, 2, TCH], F8, tag=f"hT{q}",
                                        name=f"hT{q}") for q in range(2)]
                    for q in range(2):
                        hTf = hTp.tile([128, TCH], BF16, tag=f"hTf{q}",
                                       name=f"hTf{q}")
                        nc.scalar.activation(hTf, ps_h[q], AF.Tanh,
                                             scale=1.0 / WT_SCALE)
                        # fold c in per-partition (h on partitions) so c is
                        # never itself quantized to fp8; mm2 then just
                        # partition-sums via an exact ones stationary.
                        # (DVE only: gpsimd tensor_scalar is slow+wrong on HW)
                        nc.vector.tensor_scalar_mul(
                            hT8[q][:, hh % 2, :], hTf,
                            cT_sb[:, hh, b:b + 1])
                    if hh % 2 == 0 and len(mm2_pend) > 1:
                        emit_mm2(mm2_pend.pop(0), KH // 2 - 1)
                    if hh % 2 == 1:
                        mm2_pend.append((p, hT8))

                # previous pair's pooling goes ahead of the last mm2s so
                # the final tanh + multiply have PE work to hide behind
                flush_deferred()
                for pend in mm2_pend:
                    emit_mm2(pend, KH // 2 - 1)

                # mask folded into the scores as an additive bias (host
                # passes (m-1)*50*CSCALE, so exp(s/CSCALE + bias*...) =
                # exp(s)*m to ~1e-21); exp's accum_out gives the
                # denominator for free
                for q, j in ((0, j0), (1, j1)):
                    sl = slice(j * TCH, (j + 1) * TCH)
                    nc.vector.tensor_add(ps_s[q], ps_s[q],
                                         mask_b[:, sl])
                    nc.scalar.activation(att_b[:, sl], ps_s[q],
                                         AF.Exp, scale=1.0 / CSCALE,
                                         accum_out=den_parts[:, j:j + 1])
                    deferred.append(make_pool_partial(
                        b, j, t_nat[q], att_b, ps_out, den_parts,
                        finalize=(j == NTCH - 1)))
        flush_deferred()


_NC = None


def _get_nc():
    global _NC
    if _NC is None:
        _NC = build_nc()
    return _NC


def _shard_inputs(t, a, b, mask, wt, wa, wb, wh):
    import ml_dtypes

    t32 = np.asarray(t, dtype=np.float32)
    t16 = t32.astype(ml_dtypes.bfloat16)
    # t8T[b, p, k, tau] = fp8(t[b, tau, k*128+p]), rounded once from fp32
    f8np = mybir.dt.np(F8)
    t8T = np.ascontiguousarray(
        t32.reshape(B, T, KD, 128).transpose(0, 3, 2, 1)).astype(f8np)
    a = np.asarray(a, dtype=np.float32)
    b = np.asarray(b, dtype=np.float32)
    # additive mask bias (pre-scaled by CSCALE to survive exp's 1/CSCALE):
    # exp((s + bias)/CSCALE) == exp(s/CSCALE)*m to fp32 precision
    mask_f = (np.asarray(mask).astype(np.float32) - 1.0) * 50.0 * CSCALE
    wt = np.ascontiguousarray(
        np.asarray(wt, dtype=np.float32) * np.float32(WT_SCALE))
    wa = np.ascontiguousarray(np.asarray(wa, dtype=np.float32))
    wb = np.ascontiguousarray(np.asarray(wb, dtype=np.float32))
    # CSCALE folded into wh so c lands in fp8's normal range
    wh = np.ascontiguousarray(
        np.asarray(wh, dtype=np.float32).reshape(H) * np.float32(CSCALE))
    in_maps = []
    for c in range(N_CORES):
        sl = slice(BL * c, BL * (c + 1))
        in_maps.append({
            "t": np.ascontiguousarray(t16[sl]),
            "t8": np.ascontiguousarray(t8T[sl]),
            "a": np.ascontiguousarray(a[sl]),
            "b": np.ascontiguousarray(b[sl]),
            "mask": np.ascontiguousarray(mask_f[sl]),
            "wt": wt, "wa": wa, "wb": wb, "wh": wh,
        })
    return in_maps


def kernel(t, a, b, mask, wt, wa, wb, wh):
    from concourse.bass_utils import run_bass_kernel_spmd

    nc = _get_nc()
    in_maps = _shard_inputs(t, a, b, mask, wt, wa, wb, wh)
    res = run_bass_kernel_spmd(nc, in_maps, core_ids=list(range(N_CORES)))
    out = np.concatenate([res.results[c]["out"] for c in range(N_CORES)], axis=0)
    return np.ascontiguousarray(out, dtype=np.float32)


# revision 44
# speedup vs baseline: 1.0180x; 1.0180x over previous
"""Trainium2 Bass kernel for nn_Att6 (attention-pooling block).

Computes, for each batch b:
    ht  = tanh(t[b] @ wt)                     (T, H)
    c   = tanh(a[b] @ wa) * tanh(b[b] @ wb) * wh[:, 0]        (H,)
    s   = ht @ c                              (T,)   scores
    att = softmax(s) * mask; att /= sum(att)  (T,)
    out = att @ t[b]                          (D,)

Sharding: data-parallel over batch B=32 across 8 NeuronCores (4 batches
per core), weights replicated.

Precision/layout strategy:
  - t is shipped twice from the host: native bf16 (tau on partitions,
    for pooling) and pre-transposed fp8e4m3 (d on partitions, for the
    big matmul).  No transposes on device at all.
  - mm1 (t @ wt) runs in fp8 DoubleRow perf mode (2 k-tiles per
    instruction, ~1.55x the bf16 rate incl. LDWEIGHTS overhead), with a
    1024-wide moving operand (2-bank PSUM tile) to halve the
    per-instruction LDWEIGHTS/decode overhead.  wt is pre-scaled by
    WT_SCALE on the host so its values sit in fp8's normal range; the
    tanh activation divides the scale back out.
  - mm2 (scores) also runs fp8 DoubleRow, as a partition-sum with an
    exact all-ones fp8 stationary: the c vector (pre-scaled by CSCALE,
    folded into wh on the host) is multiplied into tanh's bf16 output
    per-partition on the DVE before the fp8 cast, so c itself is never
    quantized; the exp activation divides CSCALE back out.  Measured
    end-to-end rel(max) = 1.83e-2 on HW (gate 2e-2; also 1.75e-2 on an
    independent random seed).
  - pooling stays bf16 (fp8 pooling would be 2.8e-2 > the 2e-2 gate).
"""

import sys

sys.path.insert(0, "/opt/trn_rl_repo")

import numpy as np

import bass_rust
import concourse.bass as bass
import concourse.tile as tile
from concourse import mybir
from concourse.masks import make_identity

F32 = mybir.dt.float32
F32R = mybir.dt.float32r
BF16 = mybir.dt.bfloat16
F8 = mybir.dt.float8e4
AF = mybir.ActivationFunctionType
AX = mybir.AxisListType
DR = mybir.MatmulPerfMode.DoubleRow

WT_SCALE = 32.0
CSCALE = 64.0

N_CORES = 8
B, T, D, H = 32, 2048, 1024, 1024
BL = B // N_CORES            # batches per core
TCH = 512                    # tau-chunk (pool/score granularity)
NTCH = T // TCH              # 4 chunks per batch
NPAIR = NTCH // 2            # mm1 runs on chunk pairs (1024-wide moving)
NTT = TCH // 128             # 4 tau-tiles per chunk
KD = D // 128                # 8 contraction chunks over D
KH = H // 128                # 8 chunks over H

# Ablation knobs (timing experiments only — wrong results when not default):
# pool: emit pooling matmuls; mm2: # of hh-pair score matmuls (4=all);
# mm1_k: # of DR k2 matmuls per hh (4=all)
ABLATE_DEFAULT = {"pool": True, "mm2": 4, "mm1_k": 4}
ABLATE = dict(ABLATE_DEFAULT)


def split_sync_waits(nc, max_waits=1):
    """This container's walrus accepts only one sem-wait per instruction.
    Move extra waits onto same-engine NOPs inserted immediately before."""
    n_new = 0
    for f in nc.m.functions:
        for bb in f.blocks:
            new = []
            for inst in bb.instructions:
                si = inst.sync_info
                waits = list(si.on_wait) if (si and si.on_wait) else []
                if len(waits) > max_waits:
                    extra, keep = waits[:-max_waits], waits[-max_waits:]
                    for w in extra:
                        nop = bass_rust.InstNoOp(
                            name=f"{inst.name}-sw{n_new}", ins=[], outs=[])
                        nop.engine = inst.engine
                        nop.sync_info = mybir.SyncInfo(on_wait=[w], on_update=[])
                        new.append(nop)
                        n_new += 1
                    si.on_wait = keep
                new.append(inst)
            bb.instructions[:] = new
    return n_new


def build_nc(split_waits=True, reps=1):
    nc = bass.Bass()
    t_in = nc.declare_dram_parameter("t", [BL, T, D], BF16, isOutput=False)
    # host-transposed fp8 copy: t8[b, p, k, tau] = fp8(t[b, tau, k*128+p])
    t8_in = nc.declare_dram_parameter("t8", [BL, 128, KD, T], F8,
                                      isOutput=False)
    a_in = nc.declare_dram_parameter("a", [BL, D], F32, isOutput=False)
    b_in = nc.declare_dram_parameter("b", [BL, D], F32, isOutput=False)
    m_in = nc.declare_dram_parameter("mask", [BL, T], F32, isOutput=False)
    wt_in = nc.declare_dram_parameter("wt", [D, H], F32, isOutput=False)
    wa_in = nc.declare_dram_parameter("wa", [D, H], F32, isOutput=False)
    wb_in = nc.declare_dram_parameter("wb", [D, H], F32, isOutput=False)
    wh_in = nc.declare_dram_parameter("wh", [H], F32, isOutput=False)
    out_d = nc.declare_dram_parameter("out", [BL, D], F32, isOutput=True)

    with tile.TileContext(nc) as tc:
        _body(nc, tc, t_in, t8_in, a_in, b_in, m_in, wt_in, wa_in, wb_in,
              wh_in, out_d, reps)
    if split_waits:
        split_sync_waits(nc)
    return nc


def _body(nc, tc, t_in, t8_in, a_in, b_in, m_in, wt_in, wa_in, wb_in, wh_in,
          out_d, reps):
    with (
        tc.tile_pool(name="const", bufs=1) as const,
        tc.tile_pool(name="wts", bufs=1) as wts,
        tc.tile_pool(name="wtstage", bufs=2) as wtstage,
        tc.tile_pool(name="wab", bufs=4) as wab,
        tc.tile_pool(name="small", bufs=1) as small,
        tc.tile_pool(name="tch", bufs=6) as tch,
        tc.tile_pool(name="t8b", bufs=2) as t8bp,
        tc.tile_pool(name="hT", bufs=3) as hTp,
        tc.tile_pool(name="rows", bufs=2) as rows,
        tc.tile_pool(name="rowsm", bufs=2) as rowsm,
        # PSUM budget (8 banks): mm1 3x1 + tr 1 + s 2x1 + o 2x1 = 8
        tc.tile_pool(name="ps_mm", bufs=3, space="PSUM") as ps_mm,
        tc.tile_pool(name="ps_tr", bufs=1, space="PSUM") as ps_tr,
        tc.tile_pool(name="ps_s", bufs=1, space="PSUM") as ps_sp,
        tc.tile_pool(name="ps_o", bufs=2, space="PSUM") as ps_op,
    ):
        ident = const.tile([128, 128], F32)
        make_identity(nc, ident)

        # tiny vector loads + transposes first so PE work exists early
        vT = {}
        for name, vec_in in (("a", a_in), ("b", b_in)):
            v_nat = small.tile([BL, D], F32, tag=f"v{name}")
            nc.sync.dma_start(out=v_nat, in_=vec_in[:, :])
            vT_sb = small.tile([128, KD, BL], F32R, tag=f"vT{name}")
            vT[name] = vT_sb
            for k in range(KD):
                ps = ps_tr.tile([128, BL], F32, tag="tr")
                nc.tensor.transpose(
                    ps, v_nat[:, k * 128:(k + 1) * 128], ident[:BL, :BL])
                nc.vector.tensor_copy(vT_sb[:, k, :], ps)

        def emit_chunk_dma(b, j, halves=1):
            t_nat = tch.tile([128, NTT, D], BF16, tag="tch", name="t_nat")
            hs = NTT // halves
            for h0 in range(0, NTT, hs):
                nc.sync.dma_start(
                    out=t_nat[:, h0:h0 + hs, :],
                    in_=t_in[b, (j * NTT + h0) * 128:(j * NTT + h0 + hs) * 128, :]
                    .rearrange("(tt p) d -> p tt d", p=128))
            return t_nat

        def emit_batch_t8(b):
            # host-pretransposed fp8 t: one big contiguous DMA per batch,
            # on the Activation HWDGE queue so it never queues behind the
            # native chunk stream
            t8b = t8bp.tile([128, KD, T], F8, tag="t8b", name="t8b")
            nc.scalar.dma_start(out=t8b, in_=t8_in[b])
            return t8b

        t8_batch = {0: emit_batch_t8(0)}
        nat_pre = {(0, 0): emit_chunk_dma(0, 0)}

        # weight DMA order: per h-chunk, wa/wb pair (phase 0 consumes them
        # early) interleaved with the matching wt h-slice
        w_tiles = {}
        wt_sb = wts.tile([128, KD, H], F8)
        for hh in range(KH):
            # weight DMAs ride the Activation HWDGE queue so the startup
            # weight flood never shares a queue with the chunk stream
            for name, w_in in (("a", wa_in), ("b", wb_in)):
                w_sb = wab.tile(
                    [128, KD, 128], F32R, tag="wsb", name=f"w{name}{hh}")
                nc.scalar.dma_start(
                    out=w_sb,
                    in_=w_in[:, hh * 128:(hh + 1) * 128]
                    .bitcast(F32R).rearrange("(k p) h -> p k h", p=128))
                w_tiles[(name, hh)] = w_sb
            # one-time fp8 weight quantization via a small rotating f32
            # stage (amortized across reps)
            wstage = wtstage.tile([128, KD, 128], F32R, tag="ws",
                                  name=f"ws{hh}")
            nc.scalar.dma_start(
                out=wstage,
                in_=wt_in[:, hh * 128:(hh + 1) * 128]
                .bitcast(F32R).rearrange("(k p) h -> p k h", p=128))
            nc.vector.tensor_copy(
                wt_sb[:, :, hh * 128:(hh + 1) * 128], wstage)
            if hh == 2:
                # slip batch0-chunk1's t DMA into the weight stream so its
                # data is resident when chunk0's compute finishes
                nat_pre[(0, 1)] = emit_chunk_dma(0, 1)

        whT_sb = const.tile([128, KH], F32)
        nc.scalar.dma_start(out=whT_sb,
                            in_=wh_in.rearrange("(k p) -> p k", p=128))

        # ---- phase 0 (h-chunked, interleaved into the first pair's hh
        # loop): c = tanh(a@wa) * tanh(b@wb) * wh  (wh pre-scaled by
        # CSCALE on the host); cT8 is the fp8 copy mm2 consumes ----
        cT_sb = small.tile([128, KH, BL], F32)
        # exact-1.0 fp8 stationary for the mm2 partition-sum (padded to
        # 16 cols so the DR stationary's k-tile step is 16B)
        ones8 = const.tile([128, 2, 16], F8, tag="ones8")
        nc.vector.memset(ones8, 1.0)

        def emit_phase0_hh(hh):
            hv = {}
            for name in ("a", "b"):
                w_sb = w_tiles.pop((name, hh))
                ps = ps_tr.tile([128, BL], F32, tag="tr", name="p0")
                for k in range(KD):
                    nc.tensor.matmul(
                        ps, w_sb[:, k, :], vT[name][:, k, :],
                        start=(k == 0), stop=(k == KD - 1))
                hv[name] = wab.tile(
                    [128, BL], F32, tag=f"h{name}", name=f"h{name}")
                nc.scalar.activation(hv[name], ps, AF.Tanh)
            prod = wab.tile([128, BL], F32, tag="prod")
            nc.vector.tensor_mul(prod, hv["a"], hv["b"])
            nc.vector.tensor_mul(
                cT_sb[:, hh, :], prod,
                whT_sb[:, hh:hh + 1].to_broadcast([128, BL]))

        # ---- main loop over chunk PAIRS: mm1 (1024-wide fp8 DR) ->
        # tanh (fp8 out) -> mm2 (fp8 DR over hh pairs) -> exp -> partial
        # pooling accumulate.  No score-max subtraction: |s| <= ||wh||_1
        # ~ 36 << 88, so exp cannot overflow.
        seq = [(rep, b) for rep in range(reps) for b in range(BL)]
        deferred = []

        def flush_deferred():
            while deferred:
                deferred.pop(0)()

        def make_pool_partial(b, j, t_nat, att_b, ps_out, den_parts,
                              finalize):
            def fn():
                if not ABLATE["pool"]:
                    if finalize:
                        out_b = rows.tile([1, D], F32, tag="orow",
                                          name="out_b")
                        nc.vector.tensor_copy(out_b, att_b[:, :D])
                        nc.sync.dma_start(out=out_d[b:b + 1, :], in_=out_b)
                    return
                # transpose the 4 e-columns, accumulate the pooling matmul
                attT = rowsm.tile([128, NTT], BF16, tag="attT", name="attT")
                ps_a = ps_tr.tile([128, NTT], F32, tag="tr", name="ps_a")
                for tt in range(NTT):
                    i = j * NTT + tt
                    nc.tensor.transpose(
                        ps_a[:, tt:tt + 1],
                        att_b[:, i * 128:(i + 1) * 128], ident[:1, :1])
                nc.vector.tensor_copy(attT, ps_a)
                for dh in range(2):
                    for tt in range(NTT):
                        nc.tensor.matmul(
                            ps_out[dh], attT[:, tt:tt + 1],
                            t_nat[:, tt, dh * TCH:(dh + 1) * TCH],
                            start=(j == 0 and tt == 0),
                            stop=(j == NTCH - 1 and tt == NTT - 1),
                            skip_group_check=True)
                if finalize:
                    den = rowsm.tile([1, 1], F32, tag="den", name="den")
                    nc.vector.reduce_sum(
                        out=den, in_=den_parts[:, :], axis=AX.X)
                    rden = rowsm.tile([1, 1], F32, tag="rden", name="rden")
                    nc.vector.reciprocal(rden, den)
                    out_b = rows.tile([1, D], F32, tag="orow", name="out_b")
                    for dh in range(2):
                        nc.vector.tensor_scalar_mul(
                            out_b[:, dh * TCH:(dh + 1) * TCH], ps_out[dh],
                            rden)
                    nc.sync.dma_start(out=out_d[b:b + 1, :], in_=out_b)
            return fn

        masks = {}
        for idx, (rep, b) in enumerate(seq):
            if idx in masks:
                mask_b = masks.pop(idx)
            else:
                mask_b = rows.tile([1, T], F32, tag="mrow")
                nc.sync.dma_start(out=mask_b, in_=m_in[b:b + 1, :])
            att_b = rows.tile([1, T], F32, tag="arow")
            den_parts = rowsm.tile([1, NTCH], F32, tag="denp")
            ps_out = [ps_op.tile([1, TCH], F32, tag="o", name=f"o{dh}")
                      for dh in range(2)]
            t8b = t8_batch[idx]
            for jp in range(NPAIR):
                j0, j1 = 2 * jp, 2 * jp + 1
                t_nat = [nat_pre.pop((idx, j0)), nat_pre.pop((idx, j1))]

                # --- prefetches for upcoming chunks, queued ahead of this
                # pair's PE work (Tile keeps per-engine emission order)
                for dj in range(2):
                    nxt_j = 2 * jp + 2 + dj
                    if nxt_j < NTCH:
                        nkey = (idx, nxt_j)
                    else:
                        nkey = (idx + 1, nxt_j - NTCH)
                    if nkey[0] < len(seq) and nkey not in nat_pre:
                        nat_pre[nkey] = emit_chunk_dma(seq[nkey[0]][1],
                                                       nkey[1])
                if jp == 0 and idx + 1 < len(seq) \
                        and idx + 1 not in t8_batch:
                    t8_batch[idx + 1] = emit_batch_t8(seq[idx + 1][1])
                if jp == 1 and idx + 1 < len(seq):
                    mrow = rows.tile([1, T], F32, tag="mrow")
                    nc.sync.dma_start(
                        out=mrow,
                        in_=m_in[seq[idx + 1][1]:seq[idx + 1][1] + 1, :])
                    masks[idx + 1] = mrow
                if jp == NPAIR - 1:
                    t8_batch.pop(idx, None)

                ps_s = [ps_sp.tile([1, TCH], F32, tag=f"s{q}",
                                   name=f"s{q}") for q in range(2)]
                # mm2 runs one hh-PAIR behind mm1 so each tanh has a full
                # mm1 iteration of PE time to drain before mm2 reads it
                mm2_pend = None

                def emit_mm2(pend, stop_p):
                    p, tiles = pend
                    if p >= ABLATE["mm2"]:
                        return
                    for q in range(2):
                        nc.tensor.matmul(
                            ps_s[q], ones8[:, :, 0:1], tiles[q],
                            start=(p == 0), stop=(p == stop_p),
                            perf_mode=DR, skip_group_check=True)

                for hh in range(KH):
                    nk2 = ABLATE["mm1_k"]
                    ps_h = [ps_mm.tile([128, TCH], F32, tag="mm1",
                                       name=f"mm1{q}") for q in range(2)]
                    for q, j in ((0, j0), (1, j1)):
                        sl = slice(j * TCH, (j + 1) * TCH)
                        for k2 in range(nk2):
                            nc.tensor.matmul(
                                ps_h[q],
                                wt_sb[:, 2 * k2:2 * k2 + 2,
                                      hh * 128:(hh + 1) * 128],
                                t8b[:, 2 * k2:2 * k2 + 2, sl],
                                start=(k2 == 0), stop=(k2 == nk2 - 1),
                                perf_mode=DR)
                    p = hh // 2
                    if idx == 0 and jp == 0:
                        # phase0(hh) must precede the c-multiply below,
                        # which consumes cT_sb[:, hh] in the same iteration
                        emit_phase0_hh(hh)
                    if hh % 2 == 0:
                        hT8 = [hTp.tile([128, 2, TCH], F8, tag=f"hT{q}",
                                        name=f"hT{q}") for q in range(2)]
                    for q in range(2):
                        hTf = hTp.tile([128, TCH], BF16, tag=f"hTf{q}",
                                       name=f"hTf{q}")
                        nc.scalar.activation(hTf, ps_h[q], AF.Tanh,
                                             scale=1.0 / WT_SCALE)
                        # fold c in per-partition (h on partitions) so c is
                        # never itself quantized to fp8; mm2 then just
                        # partition-sums via an exact ones stationary.
                        # (DVE only: gpsimd tensor_scalar is slow+wrong on HW)
                        nc.vector.tensor_scalar_mul(
                            hT8[q][:, hh % 2, :], hTf,
                            cT_sb[:, hh, b:b + 1])
                    if hh % 2 == 1:
                        if mm2_pend is not None:
                            emit_mm2(mm2_pend, KH // 2 - 1)
                        mm2_pend = (p, hT8)

                # previous pair's pooling goes ahead of the last mm2 so
                # the final tanh has PE work to hide behind
                flush_deferred()
                emit_mm2(mm2_pend, KH // 2 - 1)

                # mask folded into the scores as an additive bias (host
                # passes (m-1)*50*CSCALE, so exp(s/CSCALE + bias*...) =
                # exp(s)*m to ~1e-21); exp's accum_out gives the
                # denominator for free
                for q, j in ((0, j0), (1, j1)):
                    sl = slice(j * TCH, (j + 1) * TCH)
                    nc.vector.tensor_add(ps_s[q], ps_s[q],
                                         mask_b[:, sl])
                    nc.scalar.activation(att_b[:, sl], ps_s[q],
                                         AF.Exp, scale=1.0 / CSCALE,
                                         accum_out=den_parts[:, j:j + 1])
                    deferred.append(make_pool_partial(
                        b, j, t_nat[q], att_b, ps_out, den_parts,
                        finalize=(j == NTCH - 1)))
        flush_deferred()


_NC = None


def _get_nc():
    global _NC
    if _NC is None:
        _NC = build_nc()
    return _NC


def _shard_inputs(t, a, b, mask, wt, wa, wb, wh):
    import ml_dtypes

    t32 = np.asarray(t, dtype=np.float32)
    t16 = t32.astype(ml_dtypes.bfloat16)
    # t8T[b, p, k, tau] = fp8(t[b, tau, k*128+p]), rounded once from fp32
    f8np = mybir.dt.np(F8)
    t8T = np.ascontiguousarray(
        t32.reshape(B, T, KD, 128).transpose(0, 3, 2, 1)).astype(f8np)
    a = np.asarray(a, dtype=np.float32)
    b = np.asarray(b, dtype=np.float32)
    # additive mask bias (pre-scaled by CSCALE to survive exp's 1/CSCALE):
    # exp((s + bias)/CSCALE) == exp(s/CSCALE)*m to fp32 precision
    mask_f = (np.asarray(mask).astype(np.float32) - 1.0) * 50.0 * CSCALE
    wt = np.ascontiguousarray(
        np.asarray(wt, dtype=np.float32) * np.float32(WT_SCALE))
    wa = np.ascontiguousarray(np.asarray(wa, dtype=np.float32))
    wb = np.ascontiguousarray(np.asarray(wb, dtype=np.float32))
    # CSCALE folded into wh so c lands in fp8's normal range
    wh = np.ascontiguousarray(
        np.asarray(wh, dtype=np.float32).reshape(H) * np.float32(CSCALE))
    in_maps = []
    for c in range(N_CORES):
        sl = slice(BL * c, BL * (c + 1))
        in_maps.append({
            "t": np.ascontiguousarray(t16[sl]),
            "t8": np.ascontiguousarray(t8T[sl]),
            "a": np.ascontiguousarray(a[sl]),
            "b": np.ascontiguousarray(b[sl]),
            "mask": np.ascontiguousarray(mask_f[sl]),
            "wt": wt, "wa": wa, "wb": wb, "wh": wh,
        })
    return in_maps


def kernel(t, a, b, mask, wt, wa, wb, wh):
    from concourse.bass_utils import run_bass_kernel_spmd

    nc = _get_nc()
    in_maps = _shard_inputs(t, a, b, mask, wt, wa, wb, wh)
    res = run_bass_kernel_spmd(nc, in_maps, core_ids=list(range(N_CORES)))
    out = np.concatenate([res.results[c]["out"] for c in range(N_CORES)], axis=0)
    return np.ascontiguousarray(out, dtype=np.float32)


# revision 45
# speedup vs baseline: 1.0258x; 1.0076x over previous
"""Trainium2 Bass kernel for nn_Att6 (attention-pooling block).

Computes, for each batch b:
    ht  = tanh(t[b] @ wt)                     (T, H)
    c   = tanh(a[b] @ wa) * tanh(b[b] @ wb) * wh[:, 0]        (H,)
    s   = ht @ c                              (T,)   scores
    att = softmax(s) * mask; att /= sum(att)  (T,)
    out = att @ t[b]                          (D,)

Sharding: data-parallel over batch B=32 across 8 NeuronCores (4 batches
per core), weights replicated.

Precision/layout strategy:
  - t is shipped twice from the host: native bf16 (tau on partitions,
    for pooling) and pre-transposed fp8e4m3 (d on partitions, for the
    big matmul).  No transposes on device at all.
  - mm1 (t @ wt) runs in fp8 DoubleRow perf mode (2 k-tiles per
    instruction, ~1.55x the bf16 rate incl. LDWEIGHTS overhead), with a
    1024-wide moving operand (2-bank PSUM tile) to halve the
    per-instruction LDWEIGHTS/decode overhead.  wt is pre-scaled by
    WT_SCALE on the host so its values sit in fp8's normal range; the
    tanh activation divides the scale back out.
  - mm2 (scores) also runs fp8 DoubleRow, as a partition-sum with an
    exact all-ones fp8 stationary: the c vector (pre-scaled by CSCALE,
    folded into wh on the host) is multiplied into tanh's bf16 output
    per-partition on the DVE before the fp8 cast, so c itself is never
    quantized; the exp activation divides CSCALE back out.  Measured
    end-to-end rel(max) = 1.83e-2 on HW (gate 2e-2; also 1.75e-2 on an
    independent random seed).
  - pooling stays bf16 (fp8 pooling would be 2.8e-2 > the 2e-2 gate).
"""

import sys

sys.path.insert(0, "/opt/trn_rl_repo")

import numpy as np

import bass_rust
import concourse.bass as bass
import concourse.tile as tile
from concourse import mybir
from concourse.masks import make_identity

F32 = mybir.dt.float32
F32R = mybir.dt.float32r
BF16 = mybir.dt.bfloat16
F8 = mybir.dt.float8e4
AF = mybir.ActivationFunctionType
AX = mybir.AxisListType
DR = mybir.MatmulPerfMode.DoubleRow

WT_SCALE = 32.0
CSCALE = 64.0

N_CORES = 8
B, T, D, H = 32, 2048, 1024, 1024
BL = B // N_CORES            # batches per core
TCH = 512                    # tau-chunk (pool/score granularity)
NTCH = T // TCH              # 4 chunks per batch
NPAIR = NTCH // 2            # mm1 runs on chunk pairs (1024-wide moving)
NTT = TCH // 128             # 4 tau-tiles per chunk
KD = D // 128                # 8 contraction chunks over D
KH = H // 128                # 8 chunks over H

# Ablation knobs (timing experiments only — wrong results when not default):
# pool: emit pooling matmuls; mm2: # of hh-pair score matmuls (4=all);
# mm1_k: # of DR k2 matmuls per hh (4=all)
ABLATE_DEFAULT = {"pool": True, "mm2": 4, "mm1_k": 4}
ABLATE = dict(ABLATE_DEFAULT)


def split_sync_waits(nc, max_waits=1):
    """This container's walrus accepts only one sem-wait per instruction.
    Move extra waits onto same-engine NOPs inserted immediately before."""
    n_new = 0
    for f in nc.m.functions:
        for bb in f.blocks:
            new = []
            for inst in bb.instructions:
                si = inst.sync_info
                waits = list(si.on_wait) if (si and si.on_wait) else []
                if len(waits) > max_waits:
                    extra, keep = waits[:-max_waits], waits[-max_waits:]
                    for w in extra:
                        nop = bass_rust.InstNoOp(
                            name=f"{inst.name}-sw{n_new}", ins=[], outs=[])
                        nop.engine = inst.engine
                        nop.sync_info = mybir.SyncInfo(on_wait=[w], on_update=[])
                        new.append(nop)
                        n_new += 1
                    si.on_wait = keep
                new.append(inst)
            bb.instructions[:] = new
    return n_new


def build_nc(split_waits=True, reps=1):
    nc = bass.Bass()
    t_in = nc.declare_dram_parameter("t", [BL, T, D], BF16, isOutput=False)
    # host-transposed fp8 copy: t8[b, p, k, tau] = fp8(t[b, tau, k*128+p])
    t8_in = nc.declare_dram_parameter("t8", [BL, 128, KD, T], F8,
                                      isOutput=False)
    a_in = nc.declare_dram_parameter("a", [BL, D], F32, isOutput=False)
    b_in = nc.declare_dram_parameter("b", [BL, D], F32, isOutput=False)
    m_in = nc.declare_dram_parameter("mask", [BL, T], F32, isOutput=False)
    wt_in = nc.declare_dram_parameter("wt", [D, H], F32, isOutput=False)
    wa_in = nc.declare_dram_parameter("wa", [D, H], F32, isOutput=False)
    wb_in = nc.declare_dram_parameter("wb", [D, H], F32, isOutput=False)
    wh_in = nc.declare_dram_parameter("wh", [H], F32, isOutput=False)
    out_d = nc.declare_dram_parameter("out", [BL, D], F32, isOutput=True)

    with tile.TileContext(nc) as tc:
        _body(nc, tc, t_in, t8_in, a_in, b_in, m_in, wt_in, wa_in, wb_in,
              wh_in, out_d, reps)
    if split_waits:
        split_sync_waits(nc)
    return nc


def _body(nc, tc, t_in, t8_in, a_in, b_in, m_in, wt_in, wa_in, wb_in, wh_in,
          out_d, reps):
    with (
        tc.tile_pool(name="const", bufs=1) as const,
        tc.tile_pool(name="wts", bufs=1) as wts,
        tc.tile_pool(name="wtstage", bufs=2) as wtstage,
        tc.tile_pool(name="wab", bufs=4) as wab,
        tc.tile_pool(name="small", bufs=1) as small,
        tc.tile_pool(name="tch", bufs=6) as tch,
        tc.tile_pool(name="t8b", bufs=2) as t8bp,
        tc.tile_pool(name="hT", bufs=4) as hTp,
        tc.tile_pool(name="rows", bufs=2) as rows,
        tc.tile_pool(name="rowsm", bufs=2) as rowsm,
        # PSUM budget (8 banks): mm1 3x1 + tr 1 + s 2x1 + o 2x1 = 8
        tc.tile_pool(name="ps_mm", bufs=3, space="PSUM") as ps_mm,
        tc.tile_pool(name="ps_tr", bufs=1, space="PSUM") as ps_tr,
        tc.tile_pool(name="ps_s", bufs=1, space="PSUM") as ps_sp,
        tc.tile_pool(name="ps_o", bufs=2, space="PSUM") as ps_op,
    ):
        ident = const.tile([128, 128], F32)
        make_identity(nc, ident)

        # tiny vector loads + transposes first so PE work exists early
        vT = {}
        for name, vec_in in (("a", a_in), ("b", b_in)):
            v_nat = small.tile([BL, D], F32, tag=f"v{name}")
            nc.sync.dma_start(out=v_nat, in_=vec_in[:, :])
            vT_sb = small.tile([128, KD, BL], F32R, tag=f"vT{name}")
            vT[name] = vT_sb
            for k in range(KD):
                ps = ps_tr.tile([128, BL], F32, tag="tr")
                nc.tensor.transpose(
                    ps, v_nat[:, k * 128:(k + 1) * 128], ident[:BL, :BL])
                nc.vector.tensor_copy(vT_sb[:, k, :], ps)

        def emit_chunk_dma(b, j, halves=1):
            t_nat = tch.tile([128, NTT, D], BF16, tag="tch", name="t_nat")
            hs = NTT // halves
            for h0 in range(0, NTT, hs):
                nc.sync.dma_start(
                    out=t_nat[:, h0:h0 + hs, :],
                    in_=t_in[b, (j * NTT + h0) * 128:(j * NTT + h0 + hs) * 128, :]
                    .rearrange("(tt p) d -> p tt d", p=128))
            return t_nat

        def emit_batch_t8(b):
            # host-pretransposed fp8 t: one big contiguous DMA per batch,
            # on the Activation HWDGE queue so it never queues behind the
            # native chunk stream
            t8b = t8bp.tile([128, KD, T], F8, tag="t8b", name="t8b")
            nc.scalar.dma_start(out=t8b, in_=t8_in[b])
            return t8b

        t8_batch = {0: emit_batch_t8(0)}
        nat_pre = {(0, 0): emit_chunk_dma(0, 0)}

        # weight DMA order: per h-chunk, wa/wb pair (phase 0 consumes them
        # early) interleaved with the matching wt h-slice
        w_tiles = {}
        wt_sb = wts.tile([128, KD, H], F8)
        for hh in range(KH):
            # weight DMAs ride the Activation HWDGE queue so the startup
            # weight flood never shares a queue with the chunk stream
            for name, w_in in (("a", wa_in), ("b", wb_in)):
                w_sb = wab.tile(
                    [128, KD, 128], F32R, tag="wsb", name=f"w{name}{hh}")
                nc.scalar.dma_start(
                    out=w_sb,
                    in_=w_in[:, hh * 128:(hh + 1) * 128]
                    .bitcast(F32R).rearrange("(k p) h -> p k h", p=128))
                w_tiles[(name, hh)] = w_sb
            # one-time fp8 weight quantization via a small rotating f32
            # stage (amortized across reps)
            wstage = wtstage.tile([128, KD, 128], F32R, tag="ws",
                                  name=f"ws{hh}")
            nc.scalar.dma_start(
                out=wstage,
                in_=wt_in[:, hh * 128:(hh + 1) * 128]
                .bitcast(F32R).rearrange("(k p) h -> p k h", p=128))
            nc.vector.tensor_copy(
                wt_sb[:, :, hh * 128:(hh + 1) * 128], wstage)
            if hh == 2:
                # slip batch0-chunk1's t DMA into the weight stream so its
                # data is resident when chunk0's compute finishes
                nat_pre[(0, 1)] = emit_chunk_dma(0, 1)

        whT_sb = const.tile([128, KH], F32)
        nc.scalar.dma_start(out=whT_sb,
                            in_=wh_in.rearrange("(k p) -> p k", p=128))

        # ---- phase 0 (h-chunked, interleaved into the first pair's hh
        # loop): c = tanh(a@wa) * tanh(b@wb) * wh  (wh pre-scaled by
        # CSCALE on the host); cT8 is the fp8 copy mm2 consumes ----
        cT_sb = small.tile([128, KH, BL], F32)
        # exact-1.0 fp8 stationary for the mm2 partition-sum (padded to
        # 16 cols so the DR stationary's k-tile step is 16B)
        ones8 = const.tile([128, 2, 16], F8, tag="ones8")
        nc.vector.memset(ones8, 1.0)

        def emit_phase0_hh(hh):
            hv = {}
            for name in ("a", "b"):
                w_sb = w_tiles.pop((name, hh))
                ps = ps_tr.tile([128, BL], F32, tag="tr", name="p0")
                for k in range(KD):
                    nc.tensor.matmul(
                        ps, w_sb[:, k, :], vT[name][:, k, :],
                        start=(k == 0), stop=(k == KD - 1))
                hv[name] = wab.tile(
                    [128, BL], F32, tag=f"h{name}", name=f"h{name}")
                nc.scalar.activation(hv[name], ps, AF.Tanh)
            prod = wab.tile([128, BL], F32, tag="prod")
            nc.vector.tensor_mul(prod, hv["a"], hv["b"])
            nc.vector.tensor_mul(
                cT_sb[:, hh, :], prod,
                whT_sb[:, hh:hh + 1].to_broadcast([128, BL]))

        # ---- main loop over chunk PAIRS: mm1 (1024-wide fp8 DR) ->
        # tanh (fp8 out) -> mm2 (fp8 DR over hh pairs) -> exp -> partial
        # pooling accumulate.  No score-max subtraction: |s| <= ||wh||_1
        # ~ 36 << 88, so exp cannot overflow.
        seq = [(rep, b) for rep in range(reps) for b in range(BL)]
        deferred = []

        def flush_deferred():
            while deferred:
                deferred.pop(0)()

        ps_out_hold = {}

        def make_pool_partial(idx, b, j, t_nat, att_b, den_parts, finalize):
            def fn():
                if j == 0:
                    # lazy alloc at first partial: the previous batch's
                    # ring slots are already finalized, so the pool's
                    # conservative min-join release can never serialize
                    # the new accumulation against the old one
                    ps_out_hold[idx] = [
                        ps_op.tile([1, TCH], F32, tag="o", name=f"o{dh}")
                        for dh in range(2)]
                ps_out = ps_out_hold.pop(idx) if finalize \
                    else ps_out_hold[idx]
                if not ABLATE["pool"]:
                    if finalize:
                        out_b = rows.tile([1, D], F32, tag="orow",
                                          name="out_b")
                        nc.vector.tensor_copy(out_b, att_b[:, :D])
                        nc.sync.dma_start(out=out_d[b:b + 1, :], in_=out_b)
                    return
                # transpose the 4 e-columns, accumulate the pooling matmul
                attT = rowsm.tile([128, NTT], BF16, tag="attT", name="attT")
                ps_a = ps_tr.tile([128, NTT], F32, tag="tr", name="ps_a")
                for tt in range(NTT):
                    i = j * NTT + tt
                    nc.tensor.transpose(
                        ps_a[:, tt:tt + 1],
                        att_b[:, i * 128:(i + 1) * 128], ident[:1, :1])
                nc.vector.tensor_copy(attT, ps_a)
                for dh in range(2):
                    for tt in range(NTT):
                        nc.tensor.matmul(
                            ps_out[dh], attT[:, tt:tt + 1],
                            t_nat[:, tt, dh * TCH:(dh + 1) * TCH],
                            start=(j == 0 and tt == 0),
                            stop=(j == NTCH - 1 and tt == NTT - 1),
                            skip_group_check=True)
                if finalize:
                    den = rowsm.tile([1, 1], F32, tag="den", name="den")
                    nc.vector.reduce_sum(
                        out=den, in_=den_parts[:, :], axis=AX.X)
                    rden = rowsm.tile([1, 1], F32, tag="rden", name="rden")
                    nc.vector.reciprocal(rden, den)
                    out_b = rows.tile([1, D], F32, tag="orow", name="out_b")
                    for dh in range(2):
                        nc.vector.tensor_scalar_mul(
                            out_b[:, dh * TCH:(dh + 1) * TCH], ps_out[dh],
                            rden)
                    nc.sync.dma_start(out=out_d[b:b + 1, :], in_=out_b)
            return fn

        masks = {}
        for idx, (rep, b) in enumerate(seq):
            if idx in masks:
                mask_b = masks.pop(idx)
            else:
                mask_b = rows.tile([1, T], F32, tag="mrow")
                nc.sync.dma_start(out=mask_b, in_=m_in[b:b + 1, :])
            att_b = rows.tile([1, T], F32, tag="arow")
            den_parts = rowsm.tile([1, NTCH], F32, tag="denp")
            t8b = t8_batch[idx]
            for jp in range(NPAIR):
                j0, j1 = 2 * jp, 2 * jp + 1
                t_nat = [nat_pre.pop((idx, j0)), nat_pre.pop((idx, j1))]

                # --- prefetches for upcoming chunks, queued ahead of this
                # pair's PE work (Tile keeps per-engine emission order)
                for dj in range(2):
                    nxt_j = 2 * jp + 2 + dj
                    if nxt_j < NTCH:
                        nkey = (idx, nxt_j)
                    else:
                        nkey = (idx + 1, nxt_j - NTCH)
                    if nkey[0] < len(seq) and nkey not in nat_pre:
                        nat_pre[nkey] = emit_chunk_dma(seq[nkey[0]][1],
                                                       nkey[1])
                if jp == 0 and idx + 1 < len(seq) \
                        and idx + 1 not in t8_batch:
                    t8_batch[idx + 1] = emit_batch_t8(seq[idx + 1][1])
                if jp == 1 and idx + 1 < len(seq):
                    mrow = rows.tile([1, T], F32, tag="mrow")
                    nc.sync.dma_start(
                        out=mrow,
                        in_=m_in[seq[idx + 1][1]:seq[idx + 1][1] + 1, :])
                    masks[idx + 1] = mrow
                if jp == NPAIR - 1:
                    t8_batch.pop(idx, None)

                ps_s = [ps_sp.tile([1, TCH], F32, tag=f"s{q}",
                                   name=f"s{q}") for q in range(2)]
                # mm2 runs one hh-PAIR behind mm1 so each tanh has a full
                # mm1 iteration of PE time to drain before mm2 reads it
                mm2_pend = None

                def emit_mm2(pend, stop_p):
                    p, tiles = pend
                    if p >= ABLATE["mm2"]:
                        return
                    for q in range(2):
                        nc.tensor.matmul(
                            ps_s[q], ones8[:, :, 0:1], tiles[q],
                            start=(p == 0), stop=(p == stop_p),
                            perf_mode=DR, skip_group_check=True)

                for hh in range(KH):
                    nk2 = ABLATE["mm1_k"]
                    ps_h = [ps_mm.tile([128, TCH], F32, tag="mm1",
                                       name=f"mm1{q}") for q in range(2)]
                    for q, j in ((0, j0), (1, j1)):
                        sl = slice(j * TCH, (j + 1) * TCH)
                        for k2 in range(nk2):
                            nc.tensor.matmul(
                                ps_h[q],
                                wt_sb[:, 2 * k2:2 * k2 + 2,
                                      hh * 128:(hh + 1) * 128],
                                t8b[:, 2 * k2:2 * k2 + 2, sl],
                                start=(k2 == 0), stop=(k2 == nk2 - 1),
                                perf_mode=DR)
                    p = hh // 2
                    if idx == 0 and jp == 0:
                        # phase0(hh) must precede the c-multiply below,
                        # which consumes cT_sb[:, hh] in the same iteration
                        emit_phase0_hh(hh)
                    if hh % 2 == 0:
                        hT8 = [hTp.tile([128, 2, TCH], F8, tag=f"hT{q}",
                                        name=f"hT{q}") for q in range(2)]
                    for q in range(2):
                        hTf = hTp.tile([128, TCH], BF16, tag=f"hTf{q}",
                                       name=f"hTf{q}")
                        nc.scalar.activation(hTf, ps_h[q], AF.Tanh,
                                             scale=1.0 / WT_SCALE)
                        # fold c in per-partition (h on partitions) so c is
                        # never itself quantized to fp8; mm2 then just
                        # partition-sums via an exact ones stationary.
                        # (DVE only: gpsimd tensor_scalar is slow+wrong on HW)
                        nc.vector.tensor_scalar_mul(
                            hT8[q][:, hh % 2, :], hTf,
                            cT_sb[:, hh, b:b + 1])
                    if hh % 2 == 1:
                        if mm2_pend is not None:
                            emit_mm2(mm2_pend, KH // 2 - 1)
                        mm2_pend = (p, hT8)

                # previous pair's pooling goes ahead of the last mm2 so
                # the final tanh has PE work to hide behind
                flush_deferred()
                emit_mm2(mm2_pend, KH // 2 - 1)

                # mask folded into the scores as an additive bias (host
                # passes (m-1)*50*CSCALE, so exp(s/CSCALE + bias*...) =
                # exp(s)*m to ~1e-21); exp's accum_out gives the
                # denominator for free
                for q, j in ((0, j0), (1, j1)):
                    sl = slice(j * TCH, (j + 1) * TCH)
                    nc.vector.tensor_add(ps_s[q], ps_s[q],
                                         mask_b[:, sl])
                    nc.scalar.activation(att_b[:, sl], ps_s[q],
                                         AF.Exp, scale=1.0 / CSCALE,
                                         accum_out=den_parts[:, j:j + 1])
                    deferred.append(make_pool_partial(
                        idx, b, j, t_nat[q], att_b, den_parts,
                        finalize=(j == NTCH - 1)))
        flush_deferred()


_NC = None


def _get_nc():
    global _NC
    if _NC is None:
        _NC = build_nc()
    return _NC


def _shard_inputs(t, a, b, mask, wt, wa, wb, wh):
    import ml_dtypes

    t32 = np.asarray(t, dtype=np.float32)
    t16 = t32.astype(ml_dtypes.bfloat16)
    # t8T[b, p, k, tau] = fp8(t[b, tau, k*128+p]), rounded once from fp32
    f8np = mybir.dt.np(F8)
    t8T = np.ascontiguousarray(
        t32.reshape(B, T, KD, 128).transpose(0, 3, 2, 1)).astype(f8np)
    a = np.asarray(a, dtype=np.float32)
    b = np.asarray(b, dtype=np.float32)
    # additive mask bias (pre-scaled by CSCALE to survive exp's 1/CSCALE):
    # exp((s + bias)/CSCALE) == exp(s/CSCALE)*m to fp32 precision
    mask_f = (np.asarray(mask).astype(np.float32) - 1.0) * 50.0 * CSCALE
    wt = np.ascontiguousarray(
        np.asarray(wt, dtype=np.float32) * np.float32(WT_SCALE))
    wa = np.ascontiguousarray(np.asarray(wa, dtype=np.float32))
    wb = np.ascontiguousarray(np.asarray(wb, dtype=np.float32))
    # CSCALE folded into wh so c lands in fp8's normal range
    wh = np.ascontiguousarray(
        np.asarray(wh, dtype=np.float32).reshape(H) * np.float32(CSCALE))
    in_maps = []
    for c in range(N_CORES):
        sl = slice(BL * c, BL * (c + 1))
        in_maps.append({
            "t": np.ascontiguousarray(t16[sl]),
            "t8": np.ascontiguousarray(t8T[sl]),
            "a": np.ascontiguousarray(a[sl]),
            "b": np.ascontiguousarray(b[sl]),
            "mask": np.ascontiguousarray(mask_f[sl]),
            "wt": wt, "wa": wa, "wb": wb, "wh": wh,
        })
    return in_maps


def kernel(t, a, b, mask, wt, wa, wb, wh):
    from concourse.bass_utils import run_bass_kernel_spmd

    nc = _get_nc()
    in_maps = _shard_inputs(t, a, b, mask, wt, wa, wb, wh)
    res = run_bass_kernel_spmd(nc, in_maps, core_ids=list(range(N_CORES)))
    out = np.concatenate([res.results[c]["out"] for c in range(N_CORES)], axis=0)
    return np.ascontiguousarray(out, dtype=np.float32)


# revision 46
# speedup vs baseline: 1.0701x; 1.0432x over previous
"""Trainium2 Bass kernel for nn_Att6 (attention-pooling block).

Computes, for each batch b:
    ht  = tanh(t[b] @ wt)                     (T, H)
    c   = tanh(a[b] @ wa) * tanh(b[b] @ wb) * wh[:, 0]        (H,)
    s   = ht @ c                              (T,)   scores
    att = softmax(s) * mask; att /= sum(att)  (T,)
    out = att @ t[b]                          (D,)

Sharding: data-parallel over batch B=32 across 8 NeuronCores (4 batches
per core), weights replicated.

Precision/layout strategy:
  - t is shipped twice from the host: native bf16 (tau on partitions,
    for pooling) and pre-transposed fp8e4m3 (d on partitions, for the
    big matmul).  No transposes on device at all.
  - mm1 (t @ wt) runs in fp8 DoubleRow perf mode (2 k-tiles per
    instruction, ~1.55x the bf16 rate incl. LDWEIGHTS overhead), with a
    1024-wide moving operand (2-bank PSUM tile) to halve the
    per-instruction LDWEIGHTS/decode overhead.  wt is pre-scaled by
    WT_SCALE on the host so its values sit in fp8's normal range; the
    tanh activation divides the scale back out.
  - mm2 (scores) also runs fp8 DoubleRow, as a partition-sum with an
    exact all-ones fp8 stationary: the c vector (pre-scaled by CSCALE,
    folded into wh on the host) is multiplied into tanh's bf16 output
    per-partition on the DVE before the fp8 cast, so c itself is never
    quantized; the exp activation divides CSCALE back out.  Measured
    end-to-end rel(max) = 1.83e-2 on HW (gate 2e-2; also 1.75e-2 on an
    independent random seed).
  - pooling stays bf16 (fp8 pooling would be 2.8e-2 > the 2e-2 gate).
"""

import sys

sys.path.insert(0, "/opt/trn_rl_repo")

import numpy as np

import bass_rust
import concourse.bass as bass
import concourse.tile as tile
from concourse import mybir
from concourse.masks import make_identity

F32 = mybir.dt.float32
F32R = mybir.dt.float32r
BF16 = mybir.dt.bfloat16
F8 = mybir.dt.float8e4
AF = mybir.ActivationFunctionType
AX = mybir.AxisListType
DR = mybir.MatmulPerfMode.DoubleRow

WT_SCALE = 32.0
CSCALE = 64.0

N_CORES = 8
B, T, D, H = 32, 2048, 1024, 1024
BL = B // N_CORES            # batches per core
TCH = 512                    # tau-chunk (pool/score granularity)
NTCH = T // TCH              # 4 chunks per batch
NPAIR = NTCH // 2            # mm1 runs on chunk pairs (1024-wide moving)
NTT = TCH // 128             # 4 tau-tiles per chunk
KD = D // 128                # 8 contraction chunks over D
KH = H // 128                # 8 chunks over H

# Ablation knobs (timing experiments only — wrong results when not default):
# pool: emit pooling matmuls; mm2: # of hh-pair score matmuls (4=all);
# mm1_k: # of DR k2 matmuls per hh (4=all)
ABLATE_DEFAULT = {"pool": True, "mm2": 4, "mm1_k": 4}
ABLATE = dict(ABLATE_DEFAULT)


def split_sync_waits(nc, max_waits=1):
    """This container's walrus accepts only one sem-wait per instruction.
    Move extra waits onto same-engine NOPs inserted immediately before."""
    n_new = 0
    for f in nc.m.functions:
        for bb in f.blocks:
            new = []
            for inst in bb.instructions:
                si = inst.sync_info
                waits = list(si.on_wait) if (si and si.on_wait) else []
                if len(waits) > max_waits:
                    extra, keep = waits[:-max_waits], waits[-max_waits:]
                    for w in extra:
                        nop = bass_rust.InstNoOp(
                            name=f"{inst.name}-sw{n_new}", ins=[], outs=[])
                        nop.engine = inst.engine
                        nop.sync_info = mybir.SyncInfo(on_wait=[w], on_update=[])
                        new.append(nop)
                        n_new += 1
                    si.on_wait = keep
                new.append(inst)
            bb.instructions[:] = new
    return n_new


def build_nc(split_waits=True, reps=1):
    nc = bass.Bass()
    t_in = nc.declare_dram_parameter("t", [BL, T, D], BF16, isOutput=False)
    # host-transposed fp8 copy: t8[b, p, k, tau] = fp8(t[b, tau, k*128+p])
    t8_in = nc.declare_dram_parameter("t8", [BL, 128, KD, T], F8,
                                      isOutput=False)
    a_in = nc.declare_dram_parameter("a", [BL, D], F32, isOutput=False)
    b_in = nc.declare_dram_parameter("b", [BL, D], F32, isOutput=False)
    m_in = nc.declare_dram_parameter("mask", [BL, T], F32, isOutput=False)
    wt_in = nc.declare_dram_parameter("wt", [D, H], F32, isOutput=False)
    wa_in = nc.declare_dram_parameter("wa", [D, H], F32, isOutput=False)
    wb_in = nc.declare_dram_parameter("wb", [D, H], F32, isOutput=False)
    wh_in = nc.declare_dram_parameter("wh", [H], F32, isOutput=False)
    out_d = nc.declare_dram_parameter("out", [BL, D], F32, isOutput=True)

    with tile.TileContext(nc) as tc:
        _body(nc, tc, t_in, t8_in, a_in, b_in, m_in, wt_in, wa_in, wb_in,
              wh_in, out_d, reps)
    if split_waits:
        split_sync_waits(nc)
    return nc


def _body(nc, tc, t_in, t8_in, a_in, b_in, m_in, wt_in, wa_in, wb_in, wh_in,
          out_d, reps):
    with (
        tc.tile_pool(name="const", bufs=1) as const,
        tc.tile_pool(name="wts", bufs=1) as wts,
        tc.tile_pool(name="wtstage", bufs=2) as wtstage,
        tc.tile_pool(name="wab", bufs=4) as wab,
        tc.tile_pool(name="small", bufs=1) as small,
        tc.tile_pool(name="tch", bufs=6) as tch,
        tc.tile_pool(name="t8b", bufs=2) as t8bp,
        tc.tile_pool(name="hT", bufs=4) as hTp,
        tc.tile_pool(name="rows", bufs=2) as rows,
        tc.tile_pool(name="rowsm", bufs=2) as rowsm,
        # PSUM budget (8 banks): mm1 3x1 + tr 1 + s 2x1 + o 2x1 = 8
        tc.tile_pool(name="ps_mm", bufs=3, space="PSUM") as ps_mm,
        tc.tile_pool(name="ps_tr", bufs=1, space="PSUM") as ps_tr,
        tc.tile_pool(name="ps_s", bufs=1, space="PSUM") as ps_sp,
        tc.tile_pool(name="ps_o", bufs=2, space="PSUM") as ps_op,
    ):
        ident = const.tile([128, 128], F32)
        make_identity(nc, ident)

        # tiny vector loads + transposes first so PE work exists early
        vT = {}
        for name, vec_in in (("a", a_in), ("b", b_in)):
            v_nat = small.tile([BL, D], F32, tag=f"v{name}")
            nc.sync.dma_start(out=v_nat, in_=vec_in[:, :])
            vT_sb = small.tile([128, KD, BL], F32R, tag=f"vT{name}")
            vT[name] = vT_sb
            for k in range(KD):
                ps = ps_tr.tile([128, BL], F32, tag="tr")
                nc.tensor.transpose(
                    ps, v_nat[:, k * 128:(k + 1) * 128], ident[:BL, :BL])
                nc.vector.tensor_copy(vT_sb[:, k, :], ps)

        def emit_chunk_dma(b, j, halves=1):
            t_nat = tch.tile([128, NTT, D], BF16, tag="tch", name="t_nat")
            hs = NTT // halves
            for h0 in range(0, NTT, hs):
                nc.sync.dma_start(
                    out=t_nat[:, h0:h0 + hs, :],
                    in_=t_in[b, (j * NTT + h0) * 128:(j * NTT + h0 + hs) * 128, :]
                    .rearrange("(tt p) d -> p tt d", p=128))
            return t_nat

        def emit_batch_t8(b):
            # host-pretransposed fp8 t: one big contiguous DMA per batch,
            # on the Activation HWDGE queue so it never queues behind the
            # native chunk stream
            t8b = t8bp.tile([128, KD, T], F8, tag="t8b", name="t8b")
            nc.scalar.dma_start(out=t8b, in_=t8_in[b])
            return t8b

        t8_batch = {0: emit_batch_t8(0)}
        nat_pre = {(0, 0): emit_chunk_dma(0, 0)}

        # weight DMA order: per h-chunk, wa/wb pair (phase 0 consumes them
        # early) interleaved with the matching wt h-slice
        w_tiles = {}
        wt_sb = wts.tile([128, KD, H], F8)
        for hh in range(KH):
            # weight DMAs ride the Activation HWDGE queue so the startup
            # weight flood never shares a queue with the chunk stream
            for name, w_in in (("a", wa_in), ("b", wb_in)):
                w_sb = wab.tile(
                    [128, KD, 128], F32R, tag="wsb", name=f"w{name}{hh}")
                nc.scalar.dma_start(
                    out=w_sb,
                    in_=w_in[:, hh * 128:(hh + 1) * 128]
                    .bitcast(F32R).rearrange("(k p) h -> p k h", p=128))
                w_tiles[(name, hh)] = w_sb
            # one-time fp8 weight quantization via a small rotating f32
            # stage (amortized across reps)
            wstage = wtstage.tile([128, KD, 128], F32R, tag="ws",
                                  name=f"ws{hh}")
            nc.scalar.dma_start(
                out=wstage,
                in_=wt_in[:, hh * 128:(hh + 1) * 128]
                .bitcast(F32R).rearrange("(k p) h -> p k h", p=128))
            nc.vector.tensor_copy(
                wt_sb[:, :, hh * 128:(hh + 1) * 128], wstage)
            if hh == 2:
                # slip batch0-chunk1's t DMA into the weight stream so its
                # data is resident when chunk0's compute finishes
                nat_pre[(0, 1)] = emit_chunk_dma(0, 1)

        whT_sb = const.tile([128, KH], F32)
        nc.scalar.dma_start(out=whT_sb,
                            in_=wh_in.rearrange("(k p) -> p k", p=128))

        # ---- phase 0 (h-chunked, interleaved into the first pair's hh
        # loop): c = tanh(a@wa) * tanh(b@wb) * wh  (wh pre-scaled by
        # CSCALE on the host); cT8 is the fp8 copy mm2 consumes ----
        cT_sb = small.tile([128, KH, BL], F32)
        # exact-1.0 fp8 stationary for the mm2 partition-sum (padded to
        # 16 cols so the DR stationary's k-tile step is 16B)
        ones8 = const.tile([128, 2, 16], F8, tag="ones8")
        nc.vector.memset(ones8, 1.0)

        def emit_phase0_hh(hh):
            hv = {}
            for name in ("a", "b"):
                w_sb = w_tiles.pop((name, hh))
                ps = ps_tr.tile([128, BL], F32, tag="tr", name="p0")
                for k in range(KD):
                    nc.tensor.matmul(
                        ps, w_sb[:, k, :], vT[name][:, k, :],
                        start=(k == 0), stop=(k == KD - 1))
                hv[name] = wab.tile(
                    [128, BL], F32, tag=f"h{name}", name=f"h{name}")
                nc.scalar.activation(hv[name], ps, AF.Tanh)
            prod = wab.tile([128, BL], F32, tag="prod")
            nc.vector.tensor_mul(prod, hv["a"], hv["b"])
            nc.vector.tensor_mul(
                cT_sb[:, hh, :], prod,
                whT_sb[:, hh:hh + 1].to_broadcast([128, BL]))

        # ---- main loop over chunk PAIRS: mm1 (1024-wide fp8 DR) ->
        # tanh (fp8 out) -> mm2 (fp8 DR over hh pairs) -> exp -> partial
        # pooling accumulate.  No score-max subtraction: |s| <= ||wh||_1
        # ~ 36 << 88, so exp cannot overflow.
        seq = [(rep, b) for rep in range(reps) for b in range(BL)]
        deferred = []

        def flush_deferred():
            while deferred:
                deferred.pop(0)()

        ps_out_hold = {}

        def make_pool_partial(idx, b, j, t_nat, att_b, den_parts, finalize):
            def fn():
                if j == 0:
                    # lazy alloc at first partial: the previous batch's
                    # ring slots are already finalized, so the pool's
                    # conservative min-join release can never serialize
                    # the new accumulation against the old one
                    ps_out_hold[idx] = [
                        ps_op.tile([1, TCH], F32, tag="o", name=f"o{dh}")
                        for dh in range(2)]
                ps_out = ps_out_hold.pop(idx) if finalize \
                    else ps_out_hold[idx]
                if not ABLATE["pool"]:
                    if finalize:
                        out_b = rows.tile([1, D], F32, tag="orow",
                                          name="out_b")
                        nc.vector.tensor_copy(out_b, att_b[:, :D])
                        nc.sync.dma_start(out=out_d[b:b + 1, :], in_=out_b)
                    return
                # transpose the 4 e-columns, accumulate the pooling matmul
                attT = rowsm.tile([128, NTT], BF16, tag="attT", name="attT")
                ps_a = ps_tr.tile([128, NTT], F32, tag="tr", name="ps_a")
                for tt in range(NTT):
                    i = j * NTT + tt
                    nc.tensor.transpose(
                        ps_a[:, tt:tt + 1],
                        att_b[:, i * 128:(i + 1) * 128], ident[:1, :1])
                nc.vector.tensor_copy(attT, ps_a)
                for dh in range(2):
                    for tt in range(NTT):
                        nc.tensor.matmul(
                            ps_out[dh], attT[:, tt:tt + 1],
                            t_nat[:, tt, dh * TCH:(dh + 1) * TCH],
                            start=(j == 0 and tt == 0),
                            stop=(j == NTCH - 1 and tt == NTT - 1),
                            skip_group_check=True)
                if finalize:
                    den = rowsm.tile([1, 1], F32, tag="den", name="den")
                    nc.vector.reduce_sum(
                        out=den, in_=den_parts[:, :], axis=AX.X)
                    rden = rowsm.tile([1, 1], F32, tag="rden", name="rden")
                    nc.vector.reciprocal(rden, den)
                    out_b = rows.tile([1, D], F32, tag="orow", name="out_b")
                    for dh in range(2):
                        nc.vector.tensor_scalar_mul(
                            out_b[:, dh * TCH:(dh + 1) * TCH], ps_out[dh],
                            rden)
                    nc.sync.dma_start(out=out_d[b:b + 1, :], in_=out_b)
            return fn

        masks = {}
        for idx, (rep, b) in enumerate(seq):
            if idx in masks:
                mask_b = masks.pop(idx)
            else:
                mask_b = rows.tile([1, T], F32, tag="mrow")
                nc.sync.dma_start(out=mask_b, in_=m_in[b:b + 1, :])
            att_b = rows.tile([1, T], F32, tag="arow")
            den_parts = rowsm.tile([1, NTCH], F32, tag="denp")
            t8b = t8_batch[idx]
            for jp in range(NPAIR):
                j0, j1 = 2 * jp, 2 * jp + 1
                t_nat = [nat_pre.pop((idx, j0)), nat_pre.pop((idx, j1))]

                # --- prefetches for upcoming chunks, queued ahead of this
                # pair's PE work (Tile keeps per-engine emission order)
                for dj in range(2):
                    nxt_j = 2 * jp + 2 + dj
                    if nxt_j < NTCH:
                        nkey = (idx, nxt_j)
                    else:
                        nkey = (idx + 1, nxt_j - NTCH)
                    if nkey[0] < len(seq) and nkey not in nat_pre:
                        nat_pre[nkey] = emit_chunk_dma(seq[nkey[0]][1],
                                                       nkey[1])
                if jp == 0 and idx + 1 < len(seq) \
                        and idx + 1 not in t8_batch:
                    t8_batch[idx + 1] = emit_batch_t8(seq[idx + 1][1])
                if jp == 1 and idx + 1 < len(seq):
                    mrow = rows.tile([1, T], F32, tag="mrow")
                    nc.sync.dma_start(
                        out=mrow,
                        in_=m_in[seq[idx + 1][1]:seq[idx + 1][1] + 1, :])
                    masks[idx + 1] = mrow
                if jp == NPAIR - 1:
                    t8_batch.pop(idx, None)

                ps_s = [ps_sp.tile([1, TCH], F32, tag=f"s{q}",
                                   name=f"s{q}") for q in range(2)]
                # mm2 runs one hh-PAIR behind mm1 so each tanh has a full
                # mm1 iteration of PE time to drain before mm2 reads it
                mm2_pend = None

                def emit_mm2(pend, stop_p):
                    p, tiles = pend
                    if p >= ABLATE["mm2"]:
                        return
                    for q in range(2):
                        nc.tensor.matmul(
                            ps_s[q], ones8[:, :, 0:1], tiles[q],
                            start=(p == 0), stop=(p == stop_p),
                            perf_mode=DR, skip_group_check=True)

                for hh in range(KH):
                    nk2 = ABLATE["mm1_k"]
                    ps_h = [ps_mm.tile([128, TCH], F32, tag="mm1",
                                       name=f"mm1{q}") for q in range(2)]
                    for q, j in ((0, j0), (1, j1)):
                        sl = slice(j * TCH, (j + 1) * TCH)
                        for k2 in range(nk2):
                            nc.tensor.matmul(
                                ps_h[q],
                                wt_sb[:, 2 * k2:2 * k2 + 2,
                                      hh * 128:(hh + 1) * 128],
                                t8b[:, 2 * k2:2 * k2 + 2, sl],
                                start=(k2 == 0), stop=(k2 == nk2 - 1),
                                perf_mode=DR)
                    p = hh // 2
                    if idx == 0 and jp == 0:
                        # phase0(hh) must precede the c-multiply below,
                        # which consumes cT_sb[:, hh] in the same iteration
                        emit_phase0_hh(hh)
                    if hh % 2 == 0:
                        hT8 = [hTp.tile([128, 2, TCH], F8, tag=f"hT{q}",
                                        name=f"hT{q}") for q in range(2)]
                    for q in range(2):
                        # tanh in place in the PSUM bank: PSUM->PSUM access
                        # init is ~80ns cheaper than PSUM->SBUF on the
                        # ~95%-busy Act engine, and the fp32 values skip a
                        # bf16 intermediate
                        nc.scalar.activation(ps_h[q], ps_h[q], AF.Tanh,
                                             scale=1.0 / WT_SCALE)
                        # fold c in per-partition (h on partitions) so c is
                        # never itself quantized to fp8; mm2 then just
                        # partition-sums via an exact ones stationary.
                        # (DVE only: gpsimd tensor_scalar is slow+wrong on HW)
                        nc.vector.tensor_scalar_mul(
                            hT8[q][:, hh % 2, :], ps_h[q],
                            cT_sb[:, hh, b:b + 1])
                    if hh % 2 == 1:
                        if mm2_pend is not None:
                            emit_mm2(mm2_pend, KH // 2 - 1)
                        mm2_pend = (p, hT8)

                # previous pair's pooling goes ahead of the last mm2 so
                # the final tanh has PE work to hide behind
                flush_deferred()
                emit_mm2(mm2_pend, KH // 2 - 1)

                # mask folded into the scores as an additive bias (host
                # passes (m-1)*50*CSCALE, so exp(s/CSCALE + bias*...) =
                # exp(s)*m to ~1e-21); exp's accum_out gives the
                # denominator for free
                for q, j in ((0, j0), (1, j1)):
                    sl = slice(j * TCH, (j + 1) * TCH)
                    nc.vector.tensor_add(ps_s[q], ps_s[q],
                                         mask_b[:, sl])
                    nc.scalar.activation(att_b[:, sl], ps_s[q],
                                         AF.Exp, scale=1.0 / CSCALE,
                                         accum_out=den_parts[:, j:j + 1])
                    deferred.append(make_pool_partial(
                        idx, b, j, t_nat[q], att_b, den_parts,
                        finalize=(j == NTCH - 1)))
        flush_deferred()


_NC = None


def _get_nc():
    global _NC
    if _NC is None:
        _NC = build_nc()
    return _NC


def _shard_inputs(t, a, b, mask, wt, wa, wb, wh):
    import ml_dtypes

    t32 = np.asarray(t, dtype=np.float32)
    t16 = t32.astype(ml_dtypes.bfloat16)
    # t8T[b, p, k, tau] = fp8(t[b, tau, k*128+p]), rounded once from fp32
    f8np = mybir.dt.np(F8)
    t8T = np.ascontiguousarray(
        t32.reshape(B, T, KD, 128).transpose(0, 3, 2, 1)).astype(f8np)
    a = np.asarray(a, dtype=np.float32)
    b = np.asarray(b, dtype=np.float32)
    # additive mask bias (pre-scaled by CSCALE to survive exp's 1/CSCALE):
    # exp((s + bias)/CSCALE) == exp(s/CSCALE)*m to fp32 precision
    mask_f = (np.asarray(mask).astype(np.float32) - 1.0) * 50.0 * CSCALE
    wt = np.ascontiguousarray(
        np.asarray(wt, dtype=np.float32) * np.float32(WT_SCALE))
    wa = np.ascontiguousarray(np.asarray(wa, dtype=np.float32))
    wb = np.ascontiguousarray(np.asarray(wb, dtype=np.float32))
    # CSCALE folded into wh so c lands in fp8's normal range
    wh = np.ascontiguousarray(
        np.asarray(wh, dtype=np.float32).reshape(H) * np.float32(CSCALE))
    in_maps = []
    for c in range(N_CORES):
        sl = slice(BL * c, BL * (c + 1))
        in_maps.append({
            "t": np.ascontiguousarray(t16[sl]),
            "t8": np.ascontiguousarray(t8T[sl]),
            "a": np.ascontiguousarray(a[sl]),
            "b": np.ascontiguousarray(b[sl]),
            "mask": np.ascontiguousarray(mask_f[sl]),
            "wt": wt, "wa": wa, "wb": wb, "wh": wh,
        })
    return in_maps


def kernel(t, a, b, mask, wt, wa, wb, wh):
    from concourse.bass_utils import run_bass_kernel_spmd

    nc = _get_nc()
    in_maps = _shard_inputs(t, a, b, mask, wt, wa, wb, wh)
    res = run_bass_kernel_spmd(nc, in_maps, core_ids=list(range(N_CORES)))
    out = np.concatenate([res.results[c]["out"] for c in range(N_CORES)], axis=0)
    return np.ascontiguousarray(out, dtype=np.float32)
